# revision 5
# baseline (speedup 1.0000x reference)
"""BiAttention kernel for Trainium2, 8-core data-parallel SPMD.

Computes (per batch):
  x1p = relu(x1 @ W1.T + b1);  x2p = relu(x2 @ W2.T + b2)
  sim = x1p @ x2p.T  (masked with x2_mask cols / x1_mask rows)
  attn_a = rowsoftmax(sim | x2mask) @ x2
  attn_b = colsoftmax(sim | both masks).T @ x1   (all-NEG columns -> uniform mean)

Strategy: shard batch (16) across 8 cores (2 each). fp32r (TF32-rate) matmuls
for the projection/sim chain; bf16 for softmax-weight tiles and value streams.
Softmax without max-subtraction: global shift C, masks folded into ACT exp
biases (partition dim) and K=1 PSUM-bias matmuls (free dim). Row-sums via ACT
accum_out; col-sums via ones-matmuls + K=1 row->column transpose matmuls;
fully-masked columns blended to the uniform mean via an indicator K=1 matmul.
"""
import sys

sys.path.insert(0, "/opt/trn_rl_repo")

import numpy as np
import ml_dtypes

import concourse.bass as bass  # noqa: F401  (import side effects / types)
import concourse.bacc as bacc
import concourse.tile as tile
from concourse import mybir
from concourse.bass_utils import run_bass_kernel_spmd

# ---- problem constants (hardcoded per harness contract) ----
B, Nn, Mm, D = 16, 2048, 2048, 1024
NCORES = 8
BPC = B // NCORES  # batches per core
P = 128
ET, DT, NT, MT = D // P, D // P, Nn // P, Mm // P
NEG = -2e20
C_SHIFT = 75.0

F32 = mybir.dt.float32
F32R = mybir.dt.float32r
BF16 = mybir.dt.bfloat16
BF16_NP = ml_dtypes.bfloat16

Relu = mybir.ActivationFunctionType.Relu
Exp = mybir.ActivationFunctionType.Exp
Mult = mybir.AluOpType.mult


def _emit(nc):
    dram = nc.dram_tensor
    # inputs (per core)
    x1t = dram("x1t", [BPC, DT, P, Nn], F32, kind="ExternalInput")  # x1 transposed [d,n]
    x2t = dram("x2t", [BPC, DT, P, Mm], F32, kind="ExternalInput")
    w1t = dram("w1t", [DT, P, D], F32, kind="ExternalInput")  # W1.T as [d, e]
    w2t = dram("w2t", [DT, P, D], F32, kind="ExternalInput")
    b1c = dram("b1c", [P, ET], F32, kind="ExternalInput")
    b2c = dram("b2c", [P, ET], F32, kind="ExternalInput")
    x1b = dram("x1b", [BPC, NT, P, D], BF16, kind="ExternalInput")  # x1 bf16 natural
    x2b = dram("x2b", [BPC, MT, P, D], BF16, kind="ExternalInput")
    x2mbc = dram("x2mbc", [BPC, P, MT], F32, kind="ExternalInput")  # NEG*m2 - C
    x1mbc = dram("x1mbc", [BPC, P, NT], F32, kind="ExternalInput")  # NEG*m1 - C
    cb2 = dram("cb2", [BPC, 1, Mm], BF16, kind="ExternalInput")  # NEG*m2 row
    m2i = dram("m2i", [BPC, 1, Mm], BF16, kind="ExternalInput")  # m2 as 0/1 row
    blr = dram("blr", [BPC, 1, D], BF16, kind="ExternalInput")  # colsum_x1 row
    onesbf = dram("onesbf", [1, P], BF16, kind="ExternalInput")
    onescol = dram("onescol", [P, 1], BF16, kind="ExternalInput")
    one11 = dram("one11", [1, 1], F32, kind="ExternalInput")
    negc = dram("negc", [P, 1], F32, kind="ExternalInput")
    c2048 = dram("c2048", [1, 1], BF16, kind="ExternalInput")
    # outputs
    outa = dram("outa", [BPC, NT, P, D], F32, kind="ExternalOutput")
    outb = dram("outb", [BPC, MT, P, D], F32, kind="ExternalOutput")

    with tile.TileContext(nc) as tc:
        import contextlib

        with contextlib.ExitStack() as ctx:
            pp = ctx.enter_context(tc.tile_pool(name="persist", bufs=1))
            big = ctx.enter_context(tc.tile_pool(name="big32", bufs=1))
            prs = ctx.enter_context(tc.tile_pool(name="projrhs", bufs=2))
            vst = ctx.enter_context(tc.tile_pool(name="vals", bufs=3))
            stg = ctx.enter_context(tc.tile_pool(name="stage", bufs=3))
            dmp = ctx.enter_context(tc.tile_pool(name="dump", bufs=2))
            rows = ctx.enter_context(tc.tile_pool(name="rows", bufs=1))
            scr = ctx.enter_context(tc.tile_pool(name="scrow", bufs=2))
            sml = ctx.enter_context(tc.tile_pool(name="small", bufs=2))
            cst = ctx.enter_context(tc.tile_pool(name="consts", bufs=1))
            psum = ctx.enter_context(tc.tile_pool(name="psum", bufs=8, space="PSUM"))

            # constants (loaded once)
            b1c_t = cst.tile([P, ET], F32, tag="b1c")
            b2c_t = cst.tile([P, ET], F32, tag="b2c")
            onesbf_t = cst.tile([1, P], BF16, tag="onesbf")
            onescol_t = cst.tile([P, 1], BF16, tag="onescol")
            one11_t = cst.tile([1, 1], F32, tag="one11")
            c2048_t = cst.tile([1, 1], BF16, tag="c2048")
            negc_t = cst.tile([P, 1], F32, tag="negc")
            nc.sync.dma_start(out=b1c_t, in_=b1c.ap())
            nc.sync.dma_start(out=b2c_t, in_=b2c.ap())
            nc.sync.dma_start(out=onesbf_t, in_=onesbf.ap())
            nc.sync.dma_start(out=onescol_t, in_=onescol.ap())
            nc.sync.dma_start(out=one11_t, in_=one11.ap())
            nc.sync.dma_start(out=c2048_t, in_=c2048.ap())
            nc.sync.dma_start(out=negc_t, in_=negc.ap())

            for b in range(BPC):
                # ---- per-batch rows / biases ----
                x2mbc_t = sml.tile([P, MT], F32, tag="x2mbc")
                x1mbc_t = sml.tile([P, NT], F32, tag="x1mbc")
                nc.sync.dma_start(
                    out=x2mbc_t, in_=x2mbc.ap()[b : b + 1].rearrange("o p t -> p (o t)")
                )
                nc.sync.dma_start(
                    out=x1mbc_t, in_=x1mbc.ap()[b : b + 1].rearrange("o p t -> p (o t)")
                )
                cb2_t = rows.tile([1, Mm], BF16, tag="cb2")
                m2i_t = rows.tile([1, Mm], BF16, tag="m2i")
                blr_t = rows.tile([1, D], BF16, tag="blr")
                nc.sync.dma_start(
                    out=cb2_t, in_=cb2.ap()[b : b + 1].rearrange("o r m -> (o r) m")
                )
                nc.sync.dma_start(
                    out=m2i_t, in_=m2i.ap()[b : b + 1].rearrange("o r m -> (o r) m")
                )
                nc.sync.dma_start(
                    out=blr_t, in_=blr.ap()[b : b + 1].rearrange("o r m -> (o r) m")
                )
                srow_part = sml.tile([P, NT, 4], F32, tag="srp")
                srow_rec = sml.tile([P, NT], F32, tag="srr")
                scol_rec = sml.tile([P, MT], F32, tag="scr")

                # ---- PHASE P: projections ----
                x1p = pp.tile([P, ET, Nn], F32R, tag="x1p")
                x2p = pp.tile([P, ET, Mm], F32R, tag="x2p")
                for proj_out, xt, wt, bc in (
                    (x1p, x1t, w1t, b1c_t),
                    (x2p, x2t, w2t, b2c_t),
                ):
                    w_t = big.tile([P, DT, D], F32R, tag="big")
                    nc.sync.dma_start(
                        out=w_t, in_=wt.ap().rearrange("dt p e -> p dt e").bitcast(F32R)
                    )
                    NCH = 256
                    for nch in range(Nn // NCH):
                        rhs_t = prs.tile([P, DT, NCH], F32R, tag="prhs")
                        nc.sync.dma_start(
                            out=rhs_t,
                            in_=xt.ap()[b : b + 1, :, :, nch * NCH : (nch + 1) * NCH]
                            .rearrange("o dt p n -> p (o dt) n")
                            .bitcast(F32R),
                        )
                        for et in range(ET):
                            ps = psum.tile([P, 512], F32, tag="ps")
                            for dt_ in range(DT):
                                nc.tensor.matmul(
                                    ps[:, :NCH],
                                    w_t[:, dt_, et * P : (et + 1) * P],
                                    rhs_t[:, dt_, :],
                                    start=(dt_ == 0),
                                    stop=(dt_ == DT - 1),
                                )
                            nc.scalar.activation(
                                proj_out[:, et, nch * NCH : (nch + 1) * NCH],
                                ps[:, :NCH],
                                Relu,
                                bias=bc[:, et : et + 1],
                                scale=1.0,
                            )

                # ---- PHASE B: sim[n, m-half] -> F, s_row, s_col, attn_b ----
                for h in range(2):
                    f_t = big.tile([P, NT, 1024], BF16, tag="big")
                    for nt in range(NT):
                        for c2 in range(2):
                            mlo = h * 1024 + c2 * 512
                            ps = psum.tile([P, 512], F32, tag="ps")
                            for et in range(ET):
                                nc.tensor.matmul(
                                    ps,
                                    x1p[:, et, nt * P : (nt + 1) * P],
                                    x2p[:, et, mlo : mlo + 512],
                                    start=(et == 0),
                                    stop=False,
                                )
                            # + NEG on x2-masked columns (K=1, bf16)
                            nc.tensor.matmul(
                                ps,
                                onesbf_t,
                                cb2_t[0:1, mlo : mlo + 512],
                                start=False,
                                stop=True,
                                skip_group_check=True,
                            )
                            # F = exp(sim2 - C + x1maskbias)  [bf16]
                            nc.scalar.activation(
                                f_t[:, nt, c2 * 512 : (c2 + 1) * 512],
                                ps,
                                Exp,
                                bias=x1mbc_t[:, nt : nt + 1],
                                scale=1.0,
                            )
                            # s_row partial = sum_m exp(sim2 - C)  (no x1 mask)
                            dump_t = dmp.tile([P, 512], F32, tag="dump")
                            nc.scalar.activation(
                                dump_t,
                                ps,
                                Exp,
                                bias=negc_t[:, 0:1],
                                scale=1.0,
                                accum_out=srow_part[:, nt, h * 2 + c2 : h * 2 + c2 + 1],
                            )
                    # s_col for this half (col sums of F) + uniform-blend denom
                    for c2 in range(2):
                        ps_row = psum.tile([1, 512], F32, tag="ps")
                        for nt in range(NT):
                            nc.tensor.matmul(
                                ps_row,
                                onescol_t,
                                f_t[:, nt, c2 * 512 : (c2 + 1) * 512],
                                start=(nt == 0),
                                stop=(nt == NT - 1),
                            )
                        scrow_t = scr.tile([1, 512], F32, tag="scrow")
                        nc.vector.tensor_copy(scrow_t, ps_row)
                        for j in range(4):
                            mt = h * 8 + c2 * 4 + j
                            ps_sc = psum.tile([P, 1], F32, tag="ps")
                            nc.tensor.matmul(
                                ps_sc,
                                scrow_t[0:1, j * P : (j + 1) * P],
                                one11_t,
                                start=True,
                                stop=False,
                            )
                            nc.tensor.matmul(
                                ps_sc,
                                m2i_t[0:1, mt * P : (mt + 1) * P],
                                c2048_t,
                                start=False,
                                stop=True,
                                skip_group_check=True,
                            )
                            nc.vector.reciprocal(scol_rec[:, mt : mt + 1], ps_sc)
                    # attn_b for this half
                    for dch in range(2):
                        psv = [
                            psum.tile([P, 512], F32, tag="ps", name=f"psv{_j}")
                            for _j in range(8)
                        ]
                        for nt in range(NT):
                            v_t = vst.tile([P, 512], BF16, tag="vals")
                            nc.sync.dma_start(
                                out=v_t,
                                in_=x1b.ap()[
                                    b : b + 1,
                                    nt : nt + 1,
                                    :,
                                    dch * 512 : (dch + 1) * 512,
                                ].rearrange("o t p d -> p (o t d)"),
                            )
                            for j in range(8):
                                nc.tensor.matmul(
                                    psv[j],
                                    f_t[:, nt, j * P : (j + 1) * P],
                                    v_t,
                                    start=(nt == 0),
                                    stop=False,
                                )
                        for j in range(8):
                            mt = h * 8 + j
                            nc.tensor.matmul(
                                psv[j],
                                m2i_t[0:1, mt * P : (mt + 1) * P],
                                blr_t[0:1, dch * 512 : (dch + 1) * 512],
                                start=False,
                                stop=True,
                                skip_group_check=True,
                            )
                            st = stg.tile([P, 512], F32, tag="stage")
                            nc.vector.tensor_scalar(
                                out=st,
                                in0=psv[j],
                                scalar1=scol_rec[:, mt : mt + 1],
                                scalar2=None,
                                op0=Mult,
                            )
                            nc.sync.dma_start(
                                out=outb.ap()[
                                    b : b + 1,
                                    mt : mt + 1,
                                    :,
                                    dch * 512 : (dch + 1) * 512,
                                ].rearrange("o t p d -> p (o t d)"),
                                in_=st,
                            )

                # combine s_row partials, reciprocal
                nc.vector.tensor_add(srow_rec, srow_part[:, :, 0], srow_part[:, :, 1])
                nc.vector.tensor_add(srow_rec, srow_rec, srow_part[:, :, 2])
                nc.vector.tensor_add(srow_rec, srow_rec, srow_part[:, :, 3])
                nc.vector.reciprocal(srow_rec, srow_rec)

                # ---- PHASE A: simT[m, n-half] -> G, attn_a ----
                for h in range(2):
                    g_t = big.tile([P, MT, 1024], BF16, tag="big")
                    for mt in range(MT):
                        for c2 in range(2):
                            nlo = h * 1024 + c2 * 512
                            ps = psum.tile([P, 512], F32, tag="ps")
                            for et in range(ET):
                                nc.tensor.matmul(
                                    ps,
                                    x2p[:, et, mt * P : (mt + 1) * P],
                                    x1p[:, et, nlo : nlo + 512],
                                    start=(et == 0),
                                    stop=(et == ET - 1),
                                )
                            nc.scalar.activation(
                                g_t[:, mt, c2 * 512 : (c2 + 1) * 512],
                                ps,
                                Exp,
                                bias=x2mbc_t[:, mt : mt + 1],
                                scale=1.0,
                            )
                    for dch in range(2):
                        psu = [
                            psum.tile([P, 512], F32, tag="ps", name=f"psu{_j}")
                            for _j in range(8)
                        ]
                        for mt in range(MT):
                            v_t = vst.tile([P, 512], BF16, tag="vals")
                            nc.sync.dma_start(
                                out=v_t,
                                in_=x2b.ap()[
                                    b : b + 1,
                                    mt : mt + 1,
                                    :,
                                    dch * 512 : (dch + 1) * 512,
                                ].rearrange("o t p d -> p (o t d)"),
                            )
                            for j in range(8):
                                nc.tensor.matmul(
                                    psu[j],
                                    g_t[:, mt, j * P : (j + 1) * P],
                                    v_t,
                                    start=(mt == 0),
                                    stop=(mt == MT - 1),
                                )
                        for j in range(8):
                            nt = h * 8 + j
                            st = stg.tile([P, 512], F32, tag="stage")
                            nc.vector.tensor_scalar(
                                out=st,
                                in0=psu[j],
                                scalar1=srow_rec[:, nt : nt + 1],
                                scalar2=None,
                                op0=Mult,
                            )
                            nc.sync.dma_start(
                                out=outa.ap()[
                                    b : b + 1,
                                    nt : nt + 1,
                                    :,
                                    dch * 512 : (dch + 1) * 512,
                                ].rearrange("o t p d -> p (o t d)"),
                                in_=st,
                            )


_NC_CACHE = None


def _get_nc():
    global _NC_CACHE
    if _NC_CACHE is None:
        nc = bacc.Bacc("TRN2", target_bir_lowering=False, debug=False)
        _emit(nc)
        nc.compile()
        _NC_CACHE = nc
    return _NC_CACHE


def _prep_in_maps(x1, x1_mask, x2, x2_mask, W1, b1, W2, b2):
    f32 = np.float32
    x1 = np.ascontiguousarray(x1, f32)
    x2 = np.ascontiguousarray(x2, f32)
    W1 = np.ascontiguousarray(W1, f32)
    W2 = np.ascontiguousarray(W2, f32)
    b1 = np.asarray(b1, f32)
    b2 = np.asarray(b2, f32)
    m1 = np.asarray(x1_mask, bool)
    m2 = np.asarray(x2_mask, bool)

    w1t = np.ascontiguousarray(W1.T).reshape(DT, P, D)
    w2t = np.ascontiguousarray(W2.T).reshape(DT, P, D)
    b1c = np.ascontiguousarray(b1.reshape(ET, P).T)
    b2c = np.ascontiguousarray(b2.reshape(ET, P).T)
    onesbf = np.ones((1, P), BF16_NP)
    onescol = np.ones((P, 1), BF16_NP)
    one11 = np.ones((1, 1), f32)
    c2048 = np.full((1, 1), 2048.0, BF16_NP)
    negc = np.full((P, 1), -C_SHIFT, f32)

    in_maps = []
    for c in range(NCORES):
        sl = slice(c * BPC, (c + 1) * BPC)
        x1c, x2c = x1[sl], x2[sl]
        m1c, m2c = m1[sl], m2[sl]
        x1tc = np.ascontiguousarray(x1c.transpose(0, 2, 1)).reshape(BPC, DT, P, Nn)
        x2tc = np.ascontiguousarray(x2c.transpose(0, 2, 1)).reshape(BPC, DT, P, Mm)
        x1bc = np.ascontiguousarray(x1c.astype(BF16_NP)).reshape(BPC, NT, P, D)
        x2bc = np.ascontiguousarray(x2c.astype(BF16_NP)).reshape(BPC, MT, P, D)
        x2mb = np.where(m2c, np.float64(NEG), 0.0) - C_SHIFT
        x1mb = np.where(m1c, np.float64(NEG), 0.0) - C_SHIFT
        x2mbc = np.ascontiguousarray(
            x2mb.astype(f32).reshape(BPC, MT, P).transpose(0, 2, 1)
        )
        x1mbc = np.ascontiguousarray(
            x1mb.astype(f32).reshape(BPC, NT, P).transpose(0, 2, 1)
        )
        cb2 = np.where(m2c, np.float64(NEG), 0.0).astype(BF16_NP).reshape(BPC, 1, Mm)
        m2i = m2c.astype(BF16_NP).reshape(BPC, 1, Mm)
        blrow = x1c.sum(axis=1, dtype=np.float64).astype(BF16_NP).reshape(BPC, 1, D)
        in_maps.append(
            {
                "x1t": x1tc,
                "x2t": x2tc,
                "w1t": w1t,
                "w2t": w2t,
                "b1c": b1c,
                "b2c": b2c,
                "x1b": x1bc,
                "x2b": x2bc,
                "x2mbc": x2mbc,
                "x1mbc": x1mbc,
                "cb2": cb2,
                "m2i": m2i,
                "blr": blrow,
                "onesbf": onesbf,
                "onescol": onescol,
                "one11": one11,
                "negc": negc,
                "c2048": c2048,
            }
        )
    return in_maps


def kernel(x1, x1_mask, x2, x2_mask, W1, b1, W2, b2, _trace=False):
    nc = _get_nc()
    in_maps = _prep_in_maps(x1, x1_mask, x2, x2_mask, W1, b1, W2, b2)
    res = run_bass_kernel_spmd(
        nc, in_maps, core_ids=list(range(NCORES)), trace=_trace
    )
    attn_a = np.empty((B, Nn, D), np.float32)
    attn_b = np.empty((B, Mm, D), np.float32)
    for c in range(NCORES):
        sl = slice(c * BPC, (c + 1) * BPC)
        attn_a[sl] = res.results[c]["outa"].reshape(BPC, Nn, D)
        attn_b[sl] = res.results[c]["outb"].reshape(BPC, Mm, D)
    if _trace:
        kernel._last_exec_time_ns = res.exec_time_ns
        kernel._last_results = res
    return attn_a, attn_b


# revision 14
# speedup vs baseline: 52.2197x; 52.2197x over previous
"""BiAttention kernel for Trainium2, 8-core data-parallel SPMD.

Computes (per batch):
  x1p = relu(x1 @ W1.T + b1);  x2p = relu(x2 @ W2.T + b2)
  sim = x1p @ x2p.T  (masked with x2_mask cols / x1_mask rows)
  attn_a = rowsoftmax(sim | x2mask) @ x2
  attn_b = colsoftmax(sim | both masks).T @ x1   (all-NEG columns -> uniform mean)

Strategy: shard batch (16) across 8 cores (2 each). fp32r (TF32-rate) matmuls
for the projection/sim chain; bf16 softmax-weight tiles and value streams.
Single sim pass in [m, n] layout (G = exp(simT - C) with x2_mask as ACT
partition bias); the [n, m]-layout weights (F) are PE-transposes of G spilled
through a DRAM scratch. Softmax without max-subtraction via global shift C.
Row/col sums via ones-matmuls + K=1 row->column transpose matmuls; x1_mask
handled by host-zeroing x1 value rows + a keep1 column for the col-softmax
denominator; fully-masked columns blended to the uniform mean via an
indicator K=1 matmul adding [colsum_x1 | 2048] before the division.
"""
import sys

sys.path.insert(0, "/opt/trn_rl_repo")

import numpy as np
import ml_dtypes

import concourse.bass as bass  # noqa: F401
import concourse.bacc as bacc
import concourse.tile as tile
from concourse import mybir
from concourse.bass_utils import run_bass_kernel_spmd

# ---- problem constants (hardcoded per harness contract) ----
B, Nn, Mm, D = 16, 2048, 2048, 1024
NCORES = 8
BPC = B // NCORES
P = 128
ET, DT, NT, MT = D // P, D // P, Nn // P, Mm // P
NEG = -2e20
C_SHIFT = 75.0

F32 = mybir.dt.float32
F32R = mybir.dt.float32r
BF16 = mybir.dt.bfloat16
BF16_NP = ml_dtypes.bfloat16

Relu = mybir.ActivationFunctionType.Relu
Exp = mybir.ActivationFunctionType.Exp
Mult = mybir.AluOpType.mult


def _emit(nc):
    dram = nc.dram_tensor
    x1t = dram("x1t", [BPC, DT, P, Nn], F32, kind="ExternalInput")  # x1.T  [d, n]
    x2t = dram("x2t", [BPC, DT, P, Mm], F32, kind="ExternalInput")
    w1t = dram("w1t", [DT, P, D], F32, kind="ExternalInput")  # W1.T [d, e]
    w2t = dram("w2t", [DT, P, D], F32, kind="ExternalInput")
    b1c = dram("b1c", [P, ET], F32, kind="ExternalInput")
    b2c = dram("b2c", [P, ET], F32, kind="ExternalInput")
    x1b = dram("x1b", [BPC, NT, P, D], BF16, kind="ExternalInput")  # masked rows zeroed
    x2b = dram("x2b", [BPC, MT, P, D], BF16, kind="ExternalInput")
    x2mbc = dram("x2mbc", [BPC, P, MT], F32, kind="ExternalInput")  # NEG*m2 - C
    keep1c = dram("keep1c", [BPC, P, NT], BF16, kind="ExternalInput")  # ~x1_mask 0/1
    m2i = dram("m2i", [BPC, 1, Mm], BF16, kind="ExternalInput")  # m2 as 0/1 row
    blr = dram("blr", [BPC, 1, D], BF16, kind="ExternalInput")  # colsum_x1 row
    ident = dram("ident", [P, P], BF16, kind="ExternalInput")  # transpose identity
    onescol = dram("onescol", [P, 1], BF16, kind="ExternalInput")
    one11 = dram("one11", [1, 1], F32, kind="ExternalInput")
    c2048 = dram("c2048", [1, 1], BF16, kind="ExternalInput")
    outa = dram("outa", [BPC, NT, P, D], F32, kind="ExternalOutput")
    outb = dram("outb", [BPC, MT, P, D], F32, kind="ExternalOutput")

    with tile.TileContext(nc) as tc:
        import contextlib

        with contextlib.ExitStack() as ctx:
            pp = ctx.enter_context(tc.tile_pool(name="persist", bufs=1))
            big = ctx.enter_context(tc.tile_pool(name="big32", bufs=1))
            prs = ctx.enter_context(tc.tile_pool(name="projrhs", bufs=2))
            vst = ctx.enter_context(tc.tile_pool(name="vals", bufs=2))
            fts = ctx.enter_context(tc.tile_pool(name="ftstrip", bufs=3))
            stg = ctx.enter_context(tc.tile_pool(name="stage", bufs=3))
            tst = ctx.enter_context(tc.tile_pool(name="tstage", bufs=2))
            rows = ctx.enter_context(tc.tile_pool(name="rows", bufs=1))
            scr = ctx.enter_context(tc.tile_pool(name="scrow", bufs=2))
            sml = ctx.enter_context(tc.tile_pool(name="small", bufs=2))
            cst = ctx.enter_context(tc.tile_pool(name="consts", bufs=1))
            dsc = ctx.enter_context(tc.tile_pool(name="dramscr", bufs=2, space="DRAM"))
            psum = ctx.enter_context(tc.tile_pool(name="psum", bufs=8, space="PSUM"))

            # constants
            b1c_t = cst.tile([P, ET], F32, tag="b1c")
            b2c_t = cst.tile([P, ET], F32, tag="b2c")
            ident_t = cst.tile([P, P], BF16, tag="ident")
            onescol_t = cst.tile([P, 1], BF16, tag="onescol")
            one11_t = cst.tile([1, 1], F32, tag="one11")
            c2048_t = cst.tile([1, 1], BF16, tag="c2048")
            nc.sync.dma_start(out=b1c_t, in_=b1c.ap())
            nc.sync.dma_start(out=b2c_t, in_=b2c.ap())
            nc.sync.dma_start(out=ident_t, in_=ident.ap())
            nc.sync.dma_start(out=onescol_t, in_=onescol.ap())
            nc.sync.dma_start(out=one11_t, in_=one11.ap())
            nc.sync.dma_start(out=c2048_t, in_=c2048.ap())

            for b in range(BPC):
                x2mbc_t = sml.tile([P, MT], F32, tag="x2mbc")
                keep1c_t = sml.tile([P, NT], BF16, tag="keep1c")
                nc.sync.dma_start(
                    out=x2mbc_t, in_=x2mbc.ap()[b : b + 1].rearrange("o p t -> p (o t)")
                )
                nc.sync.dma_start(
                    out=keep1c_t,
                    in_=keep1c.ap()[b : b + 1].rearrange("o p t -> p (o t)"),
                )
                m2i_t = rows.tile([1, Mm], BF16, tag="m2i")
                blr_t = rows.tile([1, D], BF16, tag="blr")
                nc.sync.dma_start(
                    out=m2i_t, in_=m2i.ap()[b : b + 1].rearrange("o r m -> (o r) m")
                )
                nc.sync.dma_start(
                    out=blr_t, in_=blr.ap()[b : b + 1].rearrange("o r m -> (o r) m")
                )
                srow_rec = sml.tile([P, NT], F32, tag="srr")
                scol_rec = sml.tile([P, MT], F32, tag="scr")
                fscr = dsc.tile([NT, P, Mm], BF16, tag="fscr")  # F[n, m] scratch

                # ---- PHASE P: projections (x1p/x2p in [e, n] layout, f32r) ----
                x1p = pp.tile([P, ET, Nn], F32R, tag="x1p")
                x2p = pp.tile([P, ET, Mm], F32R, tag="x2p")
                for proj_out, xt, wt, bc in (
                    (x1p, x1t, w1t, b1c_t),
                    (x2p, x2t, w2t, b2c_t),
                ):
                    w_t = big.tile([P, DT, D], F32R, tag="big")
                    nc.sync.dma_start(
                        out=w_t, in_=wt.ap().rearrange("dt p e -> p dt e").bitcast(F32R)
                    )
                    NCH = 256
                    for nch in range(Nn // NCH):
                        rhs_t = prs.tile([P, DT, NCH], F32R, tag="prhs")
                        nc.sync.dma_start(
                            out=rhs_t,
                            in_=xt.ap()[b : b + 1, :, :, nch * NCH : (nch + 1) * NCH]
                            .rearrange("o dt p n -> p (o dt) n")
                            .bitcast(F32R),
                        )
                        for et in range(ET):
                            ps = psum.tile([P, 512], F32, tag="ps")
                            for dt_ in range(DT):
                                nc.tensor.matmul(
                                    ps[:, :NCH],
                                    w_t[:, dt_, et * P : (et + 1) * P],
                                    rhs_t[:, dt_, :],
                                    start=(dt_ == 0),
                                    stop=(dt_ == DT - 1),
                                )
                            nc.scalar.activation(
                                proj_out[:, et, nch * NCH : (nch + 1) * NCH],
                                ps[:, :NCH],
                                Relu,
                                bias=bc[:, et : et + 1],
                                scale=1.0,
                            )

                # ---- PHASE A (per n-half): simT -> G; s_row; transposes; attn_a
                for h in range(2):
                    g_t = big.tile([P, MT, 1024], BF16, tag="big")
                    for mt in range(MT):
                        for c2 in range(2):
                            nlo = h * 1024 + c2 * 512
                            ps = psum.tile([P, 512], F32, tag="ps")
                            for et in range(ET):
                                nc.tensor.matmul(
                                    ps,
                                    x2p[:, et, mt * P : (mt + 1) * P],
                                    x1p[:, et, nlo : nlo + 512],
                                    start=(et == 0),
                                    stop=(et == ET - 1),
                                )
                            nc.scalar.activation(
                                g_t[:, mt, c2 * 512 : (c2 + 1) * 512],
                                ps,
                                Exp,
                                bias=x2mbc_t[:, mt : mt + 1],
                                scale=1.0,
                            )
                    # s_row over this n-half: column sums of G (over all m)
                    sraw = sml.tile([P, 8], F32, tag="sraw")
                    for c2 in range(2):
                        ps_row = psum.tile([1, 512], F32, tag="ps")
                        for mt in range(MT):
                            nc.tensor.matmul(
                                ps_row,
                                onescol_t,
                                g_t[:, mt, c2 * 512 : (c2 + 1) * 512],
                                start=(mt == 0),
                                stop=(mt == MT - 1),
                            )
                        srow_row = scr.tile([1, 512], F32, tag="scrow")
                        nc.vector.tensor_copy(srow_row, ps_row)
                        for j in range(4):
                            ps_sr = psum.tile([P, 1], F32, tag="ps")
                            nc.tensor.matmul(
                                ps_sr,
                                srow_row[0:1, j * P : (j + 1) * P],
                                one11_t,
                                start=True,
                                stop=True,
                            )
                            nc.vector.tensor_copy(
                                sraw[:, c2 * 4 + j : c2 * 4 + j + 1], ps_sr
                            )
                    nc.vector.reciprocal(srow_rec[:, h * 8 : (h + 1) * 8], sraw)
                    # transposes: F[n, m] blocks -> DRAM scratch (one DMA per mt)
                    for mt in range(MT):
                        tst_b = tst.tile([P, 8, P], BF16, tag="tst")
                        ps_t8 = psum.tile([P, 8, P], BF16, tag="ps")
                        for ntl in range(8):
                            nc.tensor.transpose(
                                ps_t8[:, ntl, :],
                                g_t[:, mt, ntl * P : (ntl + 1) * P],
                                ident_t,
                            )
                        nc.vector.tensor_copy(tst_b, ps_t8)
                        nc.sync.dma_start(
                            out=fscr[
                                h * 8 : (h + 1) * 8, :, mt * P : (mt + 1) * P
                            ].rearrange("t p m -> p t m"),
                            in_=tst_b,
                        )
                    # attn_a for this n-half
                    for dch in range(2):
                        psu = [
                            psum.tile([P, 512], F32, tag="ps", name=f"psu{_j}")
                            for _j in range(8)
                        ]
                        for mtp in range(MT // 2):
                            v_t = vst.tile([P, 2, 512], BF16, tag="vals")
                            nc.sync.dma_start(
                                out=v_t,
                                in_=x2b.ap()[
                                    b : b + 1,
                                    2 * mtp : 2 * mtp + 2,
                                    :,
                                    dch * 512 : (dch + 1) * 512,
                                ].rearrange("o t p d -> p (o t) d"),
                            )
                            for k in range(2):
                                mt = 2 * mtp + k
                                for j in range(8):
                                    nc.tensor.matmul(
                                        psu[j],
                                        g_t[:, mt, j * P : (j + 1) * P],
                                        v_t[:, k, :],
                                        start=(mt == 0),
                                        stop=(mt == MT - 1),
                                    )
                        for j in range(8):
                            nt = h * 8 + j
                            st = stg.tile([P, 512], F32, tag="stage")
                            nc.vector.tensor_scalar(
                                out=st,
                                in0=psu[j],
                                scalar1=srow_rec[:, nt : nt + 1],
                                scalar2=None,
                                op0=Mult,
                            )
                            nc.sync.dma_start(
                                out=outa.ap()[
                                    b : b + 1,
                                    nt : nt + 1,
                                    :,
                                    dch * 512 : (dch + 1) * 512,
                                ].rearrange("o t p d -> p (o t d)"),
                                in_=st,
                            )

                # ---- PHASE B (per m-quarter): attn_b from F strips + s_col ----
                for q in range(4):
                    mq = q * 512  # m offset of this quarter
                    for dch in range(2):
                        psv = [
                            psum.tile([P, 512], F32, tag="ps", name=f"psv{_j}")
                            for _j in range(4)
                        ]
                        ps_sc = None
                        if dch == 0:
                            ps_sc = psum.tile([1, 512], F32, tag="ps", name="pssc")
                        for ntp in range(NT // 2):
                            ft_s = fts.tile([P, 2, 512], BF16, tag="fts")
                            nc.sync.dma_start(
                                out=ft_s,
                                in_=fscr[
                                    2 * ntp : 2 * ntp + 2, :, mq : mq + 512
                                ].rearrange("t p m -> p t m"),
                            )
                            v_t = vst.tile([P, 2, 512], BF16, tag="vals")
                            nc.sync.dma_start(
                                out=v_t,
                                in_=x1b.ap()[
                                    b : b + 1,
                                    2 * ntp : 2 * ntp + 2,
                                    :,
                                    dch * 512 : (dch + 1) * 512,
                                ].rearrange("o t p d -> p (o t) d"),
                            )
                            for k in range(2):
                                nt = 2 * ntp + k
                                if dch == 0:
                                    nc.tensor.matmul(
                                        ps_sc,
                                        keep1c_t[:, nt : nt + 1],
                                        ft_s[:, k, :],
                                        start=(nt == 0),
                                        stop=(nt == NT - 1),
                                    )
                                for j in range(4):
                                    nc.tensor.matmul(
                                        psv[j],
                                        ft_s[:, k, j * P : (j + 1) * P],
                                        v_t[:, k, :],
                                        start=(nt == 0),
                                        stop=False,
                                    )
                        if dch == 0:
                            scol_row = scr.tile([1, 512], F32, tag="scrow")
                            nc.vector.tensor_copy(scol_row, ps_sc)
                            scraw = sml.tile([P, 4], F32, tag="scraw")
                            for j in range(4):
                                mt = q * 4 + j
                                ps_c = psum.tile([P, 1], F32, tag="ps")
                                nc.tensor.matmul(
                                    ps_c,
                                    scol_row[0:1, j * P : (j + 1) * P],
                                    one11_t,
                                    start=True,
                                    stop=False,
                                )
                                nc.tensor.matmul(
                                    ps_c,
                                    m2i_t[0:1, mt * P : (mt + 1) * P],
                                    c2048_t,
                                    start=False,
                                    stop=True,
                                    skip_group_check=True,
                                )
                                nc.vector.tensor_copy(scraw[:, j : j + 1], ps_c)
                            nc.vector.reciprocal(
                                scol_rec[:, q * 4 : (q + 1) * 4], scraw
                            )
                        for j in range(4):
                            mt = q * 4 + j
                            nc.tensor.matmul(
                                psv[j],
                                m2i_t[0:1, mt * P : (mt + 1) * P],
                                blr_t[0:1, dch * 512 : (dch + 1) * 512],
                                start=False,
                                stop=True,
                                skip_group_check=True,
                            )
                            st = stg.tile([P, 512], F32, tag="stage")
                            nc.vector.tensor_scalar(
                                out=st,
                                in0=psv[j],
                                scalar1=scol_rec[:, mt : mt + 1],
                                scalar2=None,
                                op0=Mult,
                            )
                            nc.sync.dma_start(
                                out=outb.ap()[
                                    b : b + 1,
                                    mt : mt + 1,
                                    :,
                                    dch * 512 : (dch + 1) * 512,
                                ].rearrange("o t p d -> p (o t d)"),
                                in_=st,
                            )


_NC_CACHE = None


def _get_nc():
    global _NC_CACHE
    if _NC_CACHE is None:
        nc = bacc.Bacc("TRN2", target_bir_lowering=False, debug=False)
        _emit(nc)
        nc.compile()
        _NC_CACHE = nc
    return _NC_CACHE


def _prep_in_maps(x1, x1_mask, x2, x2_mask, W1, b1, W2, b2):
    f32 = np.float32
    x1 = np.ascontiguousarray(x1, f32)
    x2 = np.ascontiguousarray(x2, f32)
    W1 = np.ascontiguousarray(W1, f32)
    W2 = np.ascontiguousarray(W2, f32)
    b1 = np.asarray(b1, f32)
    b2 = np.asarray(b2, f32)
    m1 = np.asarray(x1_mask, bool)
    m2 = np.asarray(x2_mask, bool)

    w1t = np.ascontiguousarray(W1.T).reshape(DT, P, D)
    w2t = np.ascontiguousarray(W2.T).reshape(DT, P, D)
    b1c = np.ascontiguousarray(b1.reshape(ET, P).T)
    b2c = np.ascontiguousarray(b2.reshape(ET, P).T)
    ident = np.eye(P, dtype=BF16_NP)
    onescol = np.ones((P, 1), BF16_NP)
    one11 = np.ones((1, 1), f32)
    c2048 = np.full((1, 1), 2048.0, BF16_NP)

    in_maps = []
    for c in range(NCORES):
        sl = slice(c * BPC, (c + 1) * BPC)
        x1c, x2c = x1[sl], x2[sl]
        m1c, m2c = m1[sl], m2[sl]
        x1tc = np.ascontiguousarray(x1c.transpose(0, 2, 1)).reshape(BPC, DT, P, Nn)
        x2tc = np.ascontiguousarray(x2c.transpose(0, 2, 1)).reshape(BPC, DT, P, Mm)
        x1z = np.where(m1c[:, :, None], 0.0, x1c).astype(BF16_NP)
        x1bc = np.ascontiguousarray(x1z).reshape(BPC, NT, P, D)
        x2bc = np.ascontiguousarray(x2c.astype(BF16_NP)).reshape(BPC, MT, P, D)
        x2mb = np.where(m2c, np.float64(NEG), 0.0) - C_SHIFT
        x2mbc = np.ascontiguousarray(
            x2mb.astype(f32).reshape(BPC, MT, P).transpose(0, 2, 1)
        )
        keep1 = (~m1c).astype(BF16_NP)
        keep1c = np.ascontiguousarray(keep1.reshape(BPC, NT, P).transpose(0, 2, 1))
        m2i = m2c.astype(BF16_NP).reshape(BPC, 1, Mm)
        blrow = x1c.sum(axis=1, dtype=np.float64).astype(BF16_NP).reshape(BPC, 1, D)
        in_maps.append(
            {
                "x1t": x1tc,
                "x2t": x2tc,
                "w1t": w1t,
                "w2t": w2t,
                "b1c": b1c,
                "b2c": b2c,
                "x1b": x1bc,
                "x2b": x2bc,
                "x2mbc": x2mbc,
                "keep1c": keep1c,
                "m2i": m2i,
                "blr": blrow,
                "ident": ident,
                "onescol": onescol,
                "one11": one11,
                "c2048": c2048,
            }
        )
    return in_maps


def kernel(x1, x1_mask, x2, x2_mask, W1, b1, W2, b2, _trace=False):
    nc = _get_nc()
    in_maps = _prep_in_maps(x1, x1_mask, x2, x2_mask, W1, b1, W2, b2)
    res = run_bass_kernel_spmd(nc, in_maps, core_ids=list(range(NCORES)), trace=_trace)
    attn_a = np.empty((B, Nn, D), np.float32)
    attn_b = np.empty((B, Mm, D), np.float32)
    for c in range(NCORES):
        sl = slice(c * BPC, (c + 1) * BPC)
        attn_a[sl] = res.results[c]["outa"].reshape(BPC, Nn, D)
        attn_b[sl] = res.results[c]["outb"].reshape(BPC, Mm, D)
    if _trace:
        kernel._last_exec_time_ns = res.exec_time_ns
        kernel._last_results = res
    return attn_a, attn_b


# revision 16
# speedup vs baseline: 53.7827x; 1.0299x over previous
"""BiAttention kernel for Trainium2, 8-core data-parallel SPMD.

Computes (per batch):
  x1p = relu(x1 @ W1.T + b1);  x2p = relu(x2 @ W2.T + b2)
  sim = x1p @ x2p.T  (masked with x2_mask cols / x1_mask rows)
  attn_a = rowsoftmax(sim | x2mask) @ x2
  attn_b = colsoftmax(sim | both masks).T @ x1   (all-NEG columns -> uniform mean)

Strategy: shard batch (16) across 8 cores (2 each). fp32r (TF32-rate) matmuls
for the projection/sim chain; bf16 softmax-weight tiles and value streams.
Single sim pass in [m, n] layout (G = exp(simT - C) with x2_mask as ACT
partition bias); the [n, m]-layout weights (F) are PE-transposes of G spilled
through a DRAM scratch. Softmax without max-subtraction via global shift C.
Row/col sums via ones-matmuls + K=1 row->column transpose matmuls; x1_mask
handled by host-zeroing x1 value rows + a keep1 column for the col-softmax
denominator; fully-masked columns blended to the uniform mean via an
indicator K=1 matmul adding [colsum_x1 | 2048] before the division.
"""
import sys

sys.path.insert(0, "/opt/trn_rl_repo")

import numpy as np
import ml_dtypes

import concourse.bass as bass  # noqa: F401
import concourse.bacc as bacc
import concourse.tile as tile
from concourse import mybir
from concourse.bass_utils import run_bass_kernel_spmd

# ---- problem constants (hardcoded per harness contract) ----
B, Nn, Mm, D = 16, 2048, 2048, 1024
NCORES = 8
BPC = B // NCORES
P = 128
ET, DT, NT, MT = D // P, D // P, Nn // P, Mm // P
NEG = -2e20
C_SHIFT = 75.0

F32 = mybir.dt.float32
F32R = mybir.dt.float32r
BF16 = mybir.dt.bfloat16
BF16_NP = ml_dtypes.bfloat16

Relu = mybir.ActivationFunctionType.Relu
Exp = mybir.ActivationFunctionType.Exp
Mult = mybir.AluOpType.mult


def _emit(nc):
    dram = nc.dram_tensor
    x1t = dram("x1t", [BPC, DT, P, Nn], F32, kind="ExternalInput")  # x1.T  [d, n]
    x2t = dram("x2t", [BPC, DT, P, Mm], F32, kind="ExternalInput")
    w1t = dram("w1t", [DT, P, D], F32, kind="ExternalInput")  # W1.T [d, e]
    w2t = dram("w2t", [DT, P, D], F32, kind="ExternalInput")
    b1c = dram("b1c", [P, ET], F32, kind="ExternalInput")
    b2c = dram("b2c", [P, ET], F32, kind="ExternalInput")
    x1b = dram("x1b", [BPC, NT, P, D], BF16, kind="ExternalInput")  # masked rows zeroed
    x2b = dram("x2b", [BPC, MT, P, D], BF16, kind="ExternalInput")
    x2mbc = dram("x2mbc", [BPC, P, MT], F32, kind="ExternalInput")  # NEG*m2 - C
    keep1c = dram("keep1c", [BPC, P, NT], BF16, kind="ExternalInput")  # ~x1_mask 0/1
    m2i = dram("m2i", [BPC, 1, Mm], BF16, kind="ExternalInput")  # m2 as 0/1 row
    blr = dram("blr", [BPC, 1, D], BF16, kind="ExternalInput")  # colsum_x1 row
    ident = dram("ident", [P, P], BF16, kind="ExternalInput")  # transpose identity
    onescol = dram("onescol", [P, 1], BF16, kind="ExternalInput")
    one11 = dram("one11", [1, 1], F32, kind="ExternalInput")
    c2048 = dram("c2048", [1, 1], BF16, kind="ExternalInput")
    outa = dram("outa", [BPC, NT, P, D], F32, kind="ExternalOutput")
    outb = dram("outb", [BPC, MT, P, D], F32, kind="ExternalOutput")

    with tile.TileContext(nc) as tc:
        import contextlib

        with contextlib.ExitStack() as ctx:
            pp = ctx.enter_context(tc.tile_pool(name="persist", bufs=1))
            big = ctx.enter_context(tc.tile_pool(name="big32", bufs=1))
            prs = ctx.enter_context(tc.tile_pool(name="projrhs", bufs=2))
            vst = ctx.enter_context(tc.tile_pool(name="vals", bufs=2))
            fts = ctx.enter_context(tc.tile_pool(name="ftstrip", bufs=3))
            stg = ctx.enter_context(tc.tile_pool(name="stage", bufs=3))
            tst = ctx.enter_context(tc.tile_pool(name="tstage", bufs=2))
            rows = ctx.enter_context(tc.tile_pool(name="rows", bufs=1))
            scr = ctx.enter_context(tc.tile_pool(name="scrow", bufs=2))
            sml = ctx.enter_context(tc.tile_pool(name="small", bufs=2))
            cst = ctx.enter_context(tc.tile_pool(name="consts", bufs=1))
            dsc = ctx.enter_context(tc.tile_pool(name="dramscr", bufs=2, space="DRAM"))
            psum = ctx.enter_context(tc.tile_pool(name="psum", bufs=8, space="PSUM"))

            # constants
            b1c_t = cst.tile([P, ET], F32, tag="b1c")
            b2c_t = cst.tile([P, ET], F32, tag="b2c")
            ident_t = cst.tile([P, P], BF16, tag="ident")
            onescol_t = cst.tile([P, 1], BF16, tag="onescol")
            one11_t = cst.tile([1, 1], F32, tag="one11")
            c2048_t = cst.tile([1, 1], BF16, tag="c2048")
            nc.sync.dma_start(out=b1c_t, in_=b1c.ap())
            nc.sync.dma_start(out=b2c_t, in_=b2c.ap())
            nc.sync.dma_start(out=ident_t, in_=ident.ap())
            nc.sync.dma_start(out=onescol_t, in_=onescol.ap())
            nc.sync.dma_start(out=one11_t, in_=one11.ap())
            nc.sync.dma_start(out=c2048_t, in_=c2048.ap())

            for b in range(BPC):
                x2mbc_t = sml.tile([P, MT], F32, tag="x2mbc")
                keep1c_t = sml.tile([P, NT], BF16, tag="keep1c")
                nc.sync.dma_start(
                    out=x2mbc_t, in_=x2mbc.ap()[b : b + 1].rearrange("o p t -> p (o t)")
                )
                nc.sync.dma_start(
                    out=keep1c_t,
                    in_=keep1c.ap()[b : b + 1].rearrange("o p t -> p (o t)"),
                )
                m2i_t = rows.tile([1, Mm], BF16, tag="m2i")
                blr_t = rows.tile([1, D], BF16, tag="blr")
                nc.sync.dma_start(
                    out=m2i_t, in_=m2i.ap()[b : b + 1].rearrange("o r m -> (o r) m")
                )
                nc.sync.dma_start(
                    out=blr_t, in_=blr.ap()[b : b + 1].rearrange("o r m -> (o r) m")
                )
                srow_rec = sml.tile([P, NT], F32, tag="srr")
                scol_rec = sml.tile([P, MT], F32, tag="scr")
                fscr = dsc.tile([NT, P, Mm], BF16, tag="fscr")  # F[n, m] scratch

                # ---- PHASE P: projections (x1p/x2p in [e, n] layout, f32r) ----
                x1p = pp.tile([P, ET, Nn], F32R, tag="x1p")
                x2p = pp.tile([P, ET, Mm], F32R, tag="x2p")
                for proj_out, xt, wt, bc in (
                    (x1p, x1t, w1t, b1c_t),
                    (x2p, x2t, w2t, b2c_t),
                ):
                    w_t = big.tile([P, DT, D], F32R, tag="big")
                    for dt_ in range(DT):
                        nc.sync.dma_start(
                            out=w_t[:, dt_, :],
                            in_=wt.ap()[dt_ : dt_ + 1]
                            .rearrange("t p e -> p (t e)")
                            .bitcast(F32R),
                        )
                    NCH = 256
                    for nch in range(Nn // NCH):
                        rhs_t = prs.tile([P, DT, NCH], F32R, tag="prhs")
                        nc.sync.dma_start(
                            out=rhs_t,
                            in_=xt.ap()[b : b + 1, :, :, nch * NCH : (nch + 1) * NCH]
                            .rearrange("o dt p n -> p (o dt) n")
                            .bitcast(F32R),
                        )
                        for et in range(ET):
                            ps = psum.tile([P, 512], F32, tag="ps")
                            for dt_ in range(DT):
                                nc.tensor.matmul(
                                    ps[:, :NCH],
                                    w_t[:, dt_, et * P : (et + 1) * P],
                                    rhs_t[:, dt_, :],
                                    start=(dt_ == 0),
                                    stop=(dt_ == DT - 1),
                                )
                            nc.scalar.activation(
                                proj_out[:, et, nch * NCH : (nch + 1) * NCH],
                                ps[:, :NCH],
                                Relu,
                                bias=bc[:, et : et + 1],
                                scale=1.0,
                            )

                # ---- PHASE A (per n-half): simT -> G; s_row; transposes; attn_a
                for h in range(2):
                    g_t = big.tile([P, MT, 1024], BF16, tag="big")
                    for mt in range(MT):
                        for c2 in range(2):
                            nlo = h * 1024 + c2 * 512
                            ps = psum.tile([P, 512], F32, tag="ps")
                            for et in range(ET):
                                nc.tensor.matmul(
                                    ps,
                                    x2p[:, et, mt * P : (mt + 1) * P],
                                    x1p[:, et, nlo : nlo + 512],
                                    start=(et == 0),
                                    stop=(et == ET - 1),
                                )
                            nc.scalar.activation(
                                g_t[:, mt, c2 * 512 : (c2 + 1) * 512],
                                ps,
                                Exp,
                                bias=x2mbc_t[:, mt : mt + 1],
                                scale=1.0,
                            )
                    # s_row over this n-half: column sums of G (over all m)
                    sraw = sml.tile([P, 8], F32, tag="sraw")
                    for c2 in range(2):
                        ps_row = psum.tile([1, 512], F32, tag="ps")
                        for mt in range(MT):
                            nc.tensor.matmul(
                                ps_row,
                                onescol_t,
                                g_t[:, mt, c2 * 512 : (c2 + 1) * 512],
                                start=(mt == 0),
                                stop=(mt == MT - 1),
                            )
                        srow_row = scr.tile([1, 512], F32, tag="scrow")
                        nc.vector.tensor_copy(srow_row, ps_row)
                        for j in range(4):
                            ps_sr = psum.tile([P, 1], F32, tag="ps")
                            nc.tensor.matmul(
                                ps_sr,
                                srow_row[0:1, j * P : (j + 1) * P],
                                one11_t,
                                start=True,
                                stop=True,
                            )
                            nc.vector.tensor_copy(
                                sraw[:, c2 * 4 + j : c2 * 4 + j + 1], ps_sr
                            )
                    nc.vector.reciprocal(srow_rec[:, h * 8 : (h + 1) * 8], sraw)
                    # transposes: F[n, m] blocks -> DRAM scratch (one DMA per mt)
                    for mt in range(MT):
                        tst_b = tst.tile([P, 8, P], BF16, tag="tst")
                        ps_t8 = psum.tile([P, 8, P], BF16, tag="ps")
                        for ntl in range(8):
                            nc.tensor.transpose(
                                ps_t8[:, ntl, :],
                                g_t[:, mt, ntl * P : (ntl + 1) * P],
                                ident_t,
                            )
                        nc.vector.tensor_copy(tst_b, ps_t8)
                        nc.sync.dma_start(
                            out=fscr[
                                h * 8 : (h + 1) * 8, :, mt * P : (mt + 1) * P
                            ].rearrange("t p m -> p t m"),
                            in_=tst_b,
                        )
                    # attn_a for this n-half
                    for dch in range(2):
                        psu = [
                            psum.tile([P, 512], F32, tag="ps", name=f"psu{_j}")
                            for _j in range(8)
                        ]
                        for mtp in range(MT // 2):
                            v_t = vst.tile([P, 2, 512], BF16, tag="vals")
                            nc.sync.dma_start(
                                out=v_t,
                                in_=x2b.ap()[
                                    b : b + 1,
                                    2 * mtp : 2 * mtp + 2,
                                    :,
                                    dch * 512 : (dch + 1) * 512,
                                ].rearrange("o t p d -> p (o t) d"),
                            )
                            for k in range(2):
                                mt = 2 * mtp + k
                                for j in range(8):
                                    nc.tensor.matmul(
                                        psu[j],
                                        g_t[:, mt, j * P : (j + 1) * P],
                                        v_t[:, k, :],
                                        start=(mt == 0),
                                        stop=(mt == MT - 1),
                                    )
                        for j in range(8):
                            nt = h * 8 + j
                            st = stg.tile([P, 512], F32, tag="stage")
                            nc.vector.tensor_scalar(
                                out=st,
                                in0=psu[j],
                                scalar1=srow_rec[:, nt : nt + 1],
                                scalar2=None,
                                op0=Mult,
                            )
                            nc.sync.dma_start(
                                out=outa.ap()[
                                    b : b + 1,
                                    nt : nt + 1,
                                    :,
                                    dch * 512 : (dch + 1) * 512,
                                ].rearrange("o t p d -> p (o t d)"),
                                in_=st,
                            )

                # ---- PHASE B (per m-quarter): attn_b from F strips + s_col ----
                for q in range(4):
                    mq = q * 512  # m offset of this quarter
                    for dch in range(2):
                        psv = [
                            psum.tile([P, 512], F32, tag="ps", name=f"psv{_j}")
                            for _j in range(4)
                        ]
                        ps_sc = None
                        if dch == 0:
                            ps_sc = psum.tile([1, 512], F32, tag="ps", name="pssc")
                        for ntp in range(NT // 2):
                            ft_s = fts.tile([P, 2, 512], BF16, tag="fts")
                            nc.sync.dma_start(
                                out=ft_s,
                                in_=fscr[
                                    2 * ntp : 2 * ntp + 2, :, mq : mq + 512
                                ].rearrange("t p m -> p t m"),
                            )
                            v_t = vst.tile([P, 2, 512], BF16, tag="vals")
                            nc.sync.dma_start(
                                out=v_t,
                                in_=x1b.ap()[
                                    b : b + 1,
                                    2 * ntp : 2 * ntp + 2,
                                    :,
                                    dch * 512 : (dch + 1) * 512,
                                ].rearrange("o t p d -> p (o t) d"),
                            )
                            for k in range(2):
                                nt = 2 * ntp + k
                                if dch == 0:
                                    nc.tensor.matmul(
                                        ps_sc,
                                        keep1c_t[:, nt : nt + 1],
                                        ft_s[:, k, :],
                                        start=(nt == 0),
                                        stop=(nt == NT - 1),
                                    )
                                for j in range(4):
                                    nc.tensor.matmul(
                                        psv[j],
                                        ft_s[:, k, j * P : (j + 1) * P],
                                        v_t[:, k, :],
                                        start=(nt == 0),
                                        stop=False,
                                    )
                        if dch == 0:
                            scol_row = scr.tile([1, 512], F32, tag="scrow")
                            nc.vector.tensor_copy(scol_row, ps_sc)
                            scraw = sml.tile([P, 4], F32, tag="scraw")
                            for j in range(4):
                                mt = q * 4 + j
                                ps_c = psum.tile([P, 1], F32, tag="ps")
                                nc.tensor.matmul(
                                    ps_c,
                                    scol_row[0:1, j * P : (j + 1) * P],
                                    one11_t,
                                    start=True,
                                    stop=False,
                                )
                                nc.tensor.matmul(
                                    ps_c,
                                    m2i_t[0:1, mt * P : (mt + 1) * P],
                                    c2048_t,
                                    start=False,
                                    stop=True,
                                    skip_group_check=True,
                                )
                                nc.vector.tensor_copy(scraw[:, j : j + 1], ps_c)
                            nc.vector.reciprocal(
                                scol_rec[:, q * 4 : (q + 1) * 4], scraw
                            )
                        for j in range(4):
                            mt = q * 4 + j
                            nc.tensor.matmul(
                                psv[j],
                                m2i_t[0:1, mt * P : (mt + 1) * P],
                                blr_t[0:1, dch * 512 : (dch + 1) * 512],
                                start=False,
                                stop=True,
                                skip_group_check=True,
                            )
                            st = stg.tile([P, 512], F32, tag="stage")
                            nc.vector.tensor_scalar(
                                out=st,
                                in0=psv[j],
                                scalar1=scol_rec[:, mt : mt + 1],
                                scalar2=None,
                                op0=Mult,
                            )
                            nc.sync.dma_start(
                                out=outb.ap()[
                                    b : b + 1,
                                    mt : mt + 1,
                                    :,
                                    dch * 512 : (dch + 1) * 512,
                                ].rearrange("o t p d -> p (o t d)"),
                                in_=st,
                            )


_NC_CACHE = None


def _get_nc():
    global _NC_CACHE
    if _NC_CACHE is None:
        nc = bacc.Bacc("TRN2", target_bir_lowering=False, debug=False)
        _emit(nc)
        nc.compile()
        _NC_CACHE = nc
    return _NC_CACHE


def _prep_in_maps(x1, x1_mask, x2, x2_mask, W1, b1, W2, b2):
    f32 = np.float32
    x1 = np.ascontiguousarray(x1, f32)
    x2 = np.ascontiguousarray(x2, f32)
    W1 = np.ascontiguousarray(W1, f32)
    W2 = np.ascontiguousarray(W2, f32)
    b1 = np.asarray(b1, f32)
    b2 = np.asarray(b2, f32)
    m1 = np.asarray(x1_mask, bool)
    m2 = np.asarray(x2_mask, bool)

    w1t = np.ascontiguousarray(W1.T).reshape(DT, P, D)
    w2t = np.ascontiguousarray(W2.T).reshape(DT, P, D)
    b1c = np.ascontiguousarray(b1.reshape(ET, P).T)
    b2c = np.ascontiguousarray(b2.reshape(ET, P).T)
    ident = np.eye(P, dtype=BF16_NP)
    onescol = np.ones((P, 1), BF16_NP)
    one11 = np.ones((1, 1), f32)
    c2048 = np.full((1, 1), 2048.0, BF16_NP)

    in_maps = []
    for c in range(NCORES):
        sl = slice(c * BPC, (c + 1) * BPC)
        x1c, x2c = x1[sl], x2[sl]
        m1c, m2c = m1[sl], m2[sl]
        x1tc = np.ascontiguousarray(x1c.transpose(0, 2, 1)).reshape(BPC, DT, P, Nn)
        x2tc = np.ascontiguousarray(x2c.transpose(0, 2, 1)).reshape(BPC, DT, P, Mm)
        x1z = np.where(m1c[:, :, None], 0.0, x1c).astype(BF16_NP)
        x1bc = np.ascontiguousarray(x1z).reshape(BPC, NT, P, D)
        x2bc = np.ascontiguousarray(x2c.astype(BF16_NP)).reshape(BPC, MT, P, D)
        x2mb = np.where(m2c, np.float64(NEG), 0.0) - C_SHIFT
        x2mbc = np.ascontiguousarray(
            x2mb.astype(f32).reshape(BPC, MT, P).transpose(0, 2, 1)
        )
        keep1 = (~m1c).astype(BF16_NP)
        keep1c = np.ascontiguousarray(keep1.reshape(BPC, NT, P).transpose(0, 2, 1))
        m2i = m2c.astype(BF16_NP).reshape(BPC, 1, Mm)
        blrow = x1c.sum(axis=1, dtype=np.float64).astype(BF16_NP).reshape(BPC, 1, D)
        in_maps.append(
            {
                "x1t": x1tc,
                "x2t": x2tc,
                "w1t": w1t,
                "w2t": w2t,
                "b1c": b1c,
                "b2c": b2c,
                "x1b": x1bc,
                "x2b": x2bc,
                "x2mbc": x2mbc,
                "keep1c": keep1c,
                "m2i": m2i,
                "blr": blrow,
                "ident": ident,
                "onescol": onescol,
                "one11": one11,
                "c2048": c2048,
            }
        )
    return in_maps


def kernel(x1, x1_mask, x2, x2_mask, W1, b1, W2, b2, _trace=False):
    nc = _get_nc()
    in_maps = _prep_in_maps(x1, x1_mask, x2, x2_mask, W1, b1, W2, b2)
    res = run_bass_kernel_spmd(nc, in_maps, core_ids=list(range(NCORES)), trace=_trace)
    attn_a = np.empty((B, Nn, D), np.float32)
    attn_b = np.empty((B, Mm, D), np.float32)
    for c in range(NCORES):
        sl = slice(c * BPC, (c + 1) * BPC)
        attn_a[sl] = res.results[c]["outa"].reshape(BPC, Nn, D)
        attn_b[sl] = res.results[c]["outb"].reshape(BPC, Mm, D)
    if _trace:
        kernel._last_exec_time_ns = res.exec_time_ns
        kernel._last_results = res
    return attn_a, attn_b


# revision 17
# speedup vs baseline: 54.1272x; 1.0064x over previous
"""BiAttention kernel for Trainium2, 8-core data-parallel SPMD.

Computes (per batch):
  x1p = relu(x1 @ W1.T + b1);  x2p = relu(x2 @ W2.T + b2)
  sim = x1p @ x2p.T  (masked with x2_mask cols / x1_mask rows)
  attn_a = rowsoftmax(sim | x2mask) @ x2
  attn_b = colsoftmax(sim | both masks).T @ x1   (all-NEG columns -> uniform mean)

Strategy: shard batch (16) across 8 cores (2 each). fp32r (TF32-rate) matmuls
for the projection/sim chain; bf16 softmax-weight tiles and value streams.
Single sim pass in [m, n] layout (G = exp(simT - C) with x2_mask as ACT
partition bias); the [n, m]-layout weights (F) are PE-transposes of G spilled
through a DRAM scratch. Softmax without max-subtraction via global shift C.
Row/col sums via ones-matmuls + K=1 row->column transpose matmuls; x1_mask
handled by host-zeroing x1 value rows + a keep1 column for the col-softmax
denominator; fully-masked columns blended to the uniform mean via an
indicator K=1 matmul adding [colsum_x1 | 2048] before the division.
"""
import sys

sys.path.insert(0, "/opt/trn_rl_repo")

import numpy as np
import ml_dtypes

import concourse.bass as bass  # noqa: F401
import concourse.bacc as bacc
import concourse.tile as tile
from concourse import mybir
from concourse.bass_utils import run_bass_kernel_spmd

# ---- problem constants (hardcoded per harness contract) ----
B, Nn, Mm, D = 16, 2048, 2048, 1024
NCORES = 8
BPC = B // NCORES
P = 128
ET, DT, NT, MT = D // P, D // P, Nn // P, Mm // P
NEG = -2e20
C_SHIFT = 75.0

F32 = mybir.dt.float32
F32R = mybir.dt.float32r
BF16 = mybir.dt.bfloat16
BF16_NP = ml_dtypes.bfloat16

Relu = mybir.ActivationFunctionType.Relu
Exp = mybir.ActivationFunctionType.Exp
Mult = mybir.AluOpType.mult


def _emit(nc):
    dram = nc.dram_tensor
    x1t = dram("x1t", [BPC, DT, P, Nn], F32, kind="ExternalInput")  # x1.T  [d, n]
    x2t = dram("x2t", [BPC, DT, P, Mm], F32, kind="ExternalInput")
    w1t = dram("w1t", [DT, P, D], F32, kind="ExternalInput")  # W1.T [d, e]
    w2t = dram("w2t", [DT, P, D], F32, kind="ExternalInput")
    b1c = dram("b1c", [P, ET], F32, kind="ExternalInput")
    b2c = dram("b2c", [P, ET], F32, kind="ExternalInput")
    x1b = dram("x1b", [BPC, NT, P, D], BF16, kind="ExternalInput")  # masked rows zeroed
    x2b = dram("x2b", [BPC, MT, P, D], BF16, kind="ExternalInput")
    x2mbc = dram("x2mbc", [BPC, P, MT], F32, kind="ExternalInput")  # NEG*m2 - C
    keep1c = dram("keep1c", [BPC, P, NT], BF16, kind="ExternalInput")  # ~x1_mask 0/1
    m2i = dram("m2i", [BPC, 1, Mm], BF16, kind="ExternalInput")  # m2 as 0/1 row
    blr = dram("blr", [BPC, 1, D], BF16, kind="ExternalInput")  # colsum_x1 row
    ident = dram("ident", [P, P], BF16, kind="ExternalInput")  # transpose identity
    onescol = dram("onescol", [P, 1], BF16, kind="ExternalInput")
    one11 = dram("one11", [1, 1], F32, kind="ExternalInput")
    c2048 = dram("c2048", [1, 1], BF16, kind="ExternalInput")
    outa = dram("outa", [BPC, NT, P, D], F32, kind="ExternalOutput")
    outb = dram("outb", [BPC, MT, P, D], F32, kind="ExternalOutput")

    with tile.TileContext(nc) as tc:
        import contextlib

        with contextlib.ExitStack() as ctx:
            pp = ctx.enter_context(tc.tile_pool(name="persist", bufs=1))
            big = ctx.enter_context(tc.tile_pool(name="big32", bufs=1))
            prs = ctx.enter_context(tc.tile_pool(name="projrhs", bufs=2))
            vst = ctx.enter_context(tc.tile_pool(name="vals", bufs=2))
            fts = ctx.enter_context(tc.tile_pool(name="ftstrip", bufs=3))
            stg = ctx.enter_context(tc.tile_pool(name="stage", bufs=3))
            tst = ctx.enter_context(tc.tile_pool(name="tstage", bufs=2))
            rows = ctx.enter_context(tc.tile_pool(name="rows", bufs=1))
            scr = ctx.enter_context(tc.tile_pool(name="scrow", bufs=2))
            sml = ctx.enter_context(tc.tile_pool(name="small", bufs=2))
            cst = ctx.enter_context(tc.tile_pool(name="consts", bufs=1))
            dsc = ctx.enter_context(tc.tile_pool(name="dramscr", bufs=2, space="DRAM"))
            psum = ctx.enter_context(tc.tile_pool(name="psum", bufs=8, space="PSUM"))

            # constants
            b1c_t = cst.tile([P, ET], F32, tag="b1c")
            b2c_t = cst.tile([P, ET], F32, tag="b2c")
            ident_t = cst.tile([P, P], BF16, tag="ident")
            onescol_t = cst.tile([P, 1], BF16, tag="onescol")
            one11_t = cst.tile([1, 1], F32, tag="one11")
            c2048_t = cst.tile([1, 1], BF16, tag="c2048")
            nc.sync.dma_start(out=b1c_t, in_=b1c.ap())
            nc.sync.dma_start(out=b2c_t, in_=b2c.ap())
            nc.sync.dma_start(out=ident_t, in_=ident.ap())
            nc.sync.dma_start(out=onescol_t, in_=onescol.ap())
            nc.sync.dma_start(out=one11_t, in_=one11.ap())
            nc.sync.dma_start(out=c2048_t, in_=c2048.ap())

            for b in range(BPC):
                x2mbc_t = sml.tile([P, MT], F32, tag="x2mbc")
                keep1c_t = sml.tile([P, NT], BF16, tag="keep1c")
                nc.sync.dma_start(
                    out=x2mbc_t, in_=x2mbc.ap()[b : b + 1].rearrange("o p t -> p (o t)")
                )
                nc.sync.dma_start(
                    out=keep1c_t,
                    in_=keep1c.ap()[b : b + 1].rearrange("o p t -> p (o t)"),
                )
                m2i_t = rows.tile([1, Mm], BF16, tag="m2i")
                blr_t = rows.tile([1, D], BF16, tag="blr")
                nc.sync.dma_start(
                    out=m2i_t, in_=m2i.ap()[b : b + 1].rearrange("o r m -> (o r) m")
                )
                nc.sync.dma_start(
                    out=blr_t, in_=blr.ap()[b : b + 1].rearrange("o r m -> (o r) m")
                )
                srow_rec = sml.tile([P, NT], F32, tag="srr")
                scol_rec = sml.tile([P, MT], F32, tag="scr")
                fscr = dsc.tile([NT, P, Mm], BF16, tag="fscr")  # F[n, m] scratch

                # ---- PHASE P: projections (x1p/x2p in [e, n] layout, f32r) ----
                x1p = pp.tile([P, ET, Nn], F32R, tag="x1p")
                x2p = pp.tile([P, ET, Mm], F32R, tag="x2p")
                for proj_out, xt, wt, bc in (
                    (x1p, x1t, w1t, b1c_t),
                    (x2p, x2t, w2t, b2c_t),
                ):
                    w_t = big.tile([P, DT, D], F32R, tag="big")
                    for dt_ in range(DT):
                        nc.sync.dma_start(
                            out=w_t[:, dt_, :],
                            in_=wt.ap()[dt_ : dt_ + 1]
                            .rearrange("t p e -> p (t e)")
                            .bitcast(F32R),
                        )
                    NCH = 256
                    for nch in range(Nn // NCH):
                        rhs_t = prs.tile([P, DT, NCH], F32R, tag="prhs")
                        for dt_ in range(0, DT, 2):
                            nc.sync.dma_start(
                                out=rhs_t[:, dt_ : dt_ + 2, :],
                                in_=xt.ap()[
                                    b : b + 1,
                                    dt_ : dt_ + 2,
                                    :,
                                    nch * NCH : (nch + 1) * NCH,
                                ]
                                .rearrange("o dt p n -> p (o dt) n")
                                .bitcast(F32R),
                            )
                        for et in range(ET):
                            ps = psum.tile([P, 512], F32, tag="ps")
                            for dt_ in range(DT):
                                nc.tensor.matmul(
                                    ps[:, :NCH],
                                    w_t[:, dt_, et * P : (et + 1) * P],
                                    rhs_t[:, dt_, :],
                                    start=(dt_ == 0),
                                    stop=(dt_ == DT - 1),
                                )
                            nc.scalar.activation(
                                proj_out[:, et, nch * NCH : (nch + 1) * NCH],
                                ps[:, :NCH],
                                Relu,
                                bias=bc[:, et : et + 1],
                                scale=1.0,
                            )

                # ---- PHASE A (per n-half): simT -> G; s_row; transposes; attn_a
                for h in range(2):
                    g_t = big.tile([P, MT, 1024], BF16, tag="big")
                    for mt in range(MT):
                        for c2 in range(2):
                            nlo = h * 1024 + c2 * 512
                            ps = psum.tile([P, 512], F32, tag="ps")
                            for et in range(ET):
                                nc.tensor.matmul(
                                    ps,
                                    x2p[:, et, mt * P : (mt + 1) * P],
                                    x1p[:, et, nlo : nlo + 512],
                                    start=(et == 0),
                                    stop=(et == ET - 1),
                                )
                            nc.scalar.activation(
                                g_t[:, mt, c2 * 512 : (c2 + 1) * 512],
                                ps,
                                Exp,
                                bias=x2mbc_t[:, mt : mt + 1],
                                scale=1.0,
                            )
                    # s_row over this n-half: column sums of G (over all m)
                    sraw = sml.tile([P, 8], F32, tag="sraw")
                    for c2 in range(2):
                        ps_row = psum.tile([1, 512], F32, tag="ps")
                        for mt in range(MT):
                            nc.tensor.matmul(
                                ps_row,
                                onescol_t,
                                g_t[:, mt, c2 * 512 : (c2 + 1) * 512],
                                start=(mt == 0),
                                stop=(mt == MT - 1),
                            )
                        srow_row = scr.tile([1, 512], F32, tag="scrow")
                        nc.vector.tensor_copy(srow_row, ps_row)
                        for j in range(4):
                            ps_sr = psum.tile([P, 1], F32, tag="ps")
                            nc.tensor.matmul(
                                ps_sr,
                                srow_row[0:1, j * P : (j + 1) * P],
                                one11_t,
                                start=True,
                                stop=True,
                            )
                            nc.vector.tensor_copy(
                                sraw[:, c2 * 4 + j : c2 * 4 + j + 1], ps_sr
                            )
                    nc.vector.reciprocal(srow_rec[:, h * 8 : (h + 1) * 8], sraw)
                    # transposes: F[n, m] blocks -> DRAM scratch (one DMA per mt)
                    for mt in range(MT):
                        tst_b = tst.tile([P, 8, P], BF16, tag="tst")
                        ps_t8 = psum.tile([P, 8, P], BF16, tag="ps")
                        for ntl in range(8):
                            nc.tensor.transpose(
                                ps_t8[:, ntl, :],
                                g_t[:, mt, ntl * P : (ntl + 1) * P],
                                ident_t,
                            )
                        nc.vector.tensor_copy(tst_b, ps_t8)
                        nc.sync.dma_start(
                            out=fscr[
                                h * 8 : (h + 1) * 8, :, mt * P : (mt + 1) * P
                            ].rearrange("t p m -> p t m"),
                            in_=tst_b,
                        )
                    # attn_a for this n-half
                    for dch in range(2):
                        psu = [
                            psum.tile([P, 512], F32, tag="ps", name=f"psu{_j}")
                            for _j in range(8)
                        ]
                        for mtp in range(MT // 2):
                            v_t = vst.tile([P, 2, 512], BF16, tag="vals")
                            nc.sync.dma_start(
                                out=v_t,
                                in_=x2b.ap()[
                                    b : b + 1,
                                    2 * mtp : 2 * mtp + 2,
                                    :,
                                    dch * 512 : (dch + 1) * 512,
                                ].rearrange("o t p d -> p (o t) d"),
                            )
                            for k in range(2):
                                mt = 2 * mtp + k
                                for j in range(8):
                                    nc.tensor.matmul(
                                        psu[j],
                                        g_t[:, mt, j * P : (j + 1) * P],
                                        v_t[:, k, :],
                                        start=(mt == 0),
                                        stop=(mt == MT - 1),
                                    )
                        for j in range(8):
                            nt = h * 8 + j
                            st = stg.tile([P, 512], F32, tag="stage")
                            nc.vector.tensor_scalar(
                                out=st,
                                in0=psu[j],
                                scalar1=srow_rec[:, nt : nt + 1],
                                scalar2=None,
                                op0=Mult,
                            )
                            nc.sync.dma_start(
                                out=outa.ap()[
                                    b : b + 1,
                                    nt : nt + 1,
                                    :,
                                    dch * 512 : (dch + 1) * 512,
                                ].rearrange("o t p d -> p (o t d)"),
                                in_=st,
                            )

                # ---- PHASE B (per m-quarter): attn_b from F strips + s_col ----
                for q in range(4):
                    mq = q * 512  # m offset of this quarter
                    for dch in range(2):
                        psv = [
                            psum.tile([P, 512], F32, tag="ps", name=f"psv{_j}")
                            for _j in range(4)
                        ]
                        ps_sc = None
                        if dch == 0:
                            ps_sc = psum.tile([1, 512], F32, tag="ps", name="pssc")
                        for ntp in range(NT // 2):
                            ft_s = fts.tile([P, 2, 512], BF16, tag="fts")
                            nc.sync.dma_start(
                                out=ft_s,
                                in_=fscr[
                                    2 * ntp : 2 * ntp + 2, :, mq : mq + 512
                                ].rearrange("t p m -> p t m"),
                            )
                            v_t = vst.tile([P, 2, 512], BF16, tag="vals")
                            nc.sync.dma_start(
                                out=v_t,
                                in_=x1b.ap()[
                                    b : b + 1,
                                    2 * ntp : 2 * ntp + 2,
                                    :,
                                    dch * 512 : (dch + 1) * 512,
                                ].rearrange("o t p d -> p (o t) d"),
                            )
                            for k in range(2):
                                nt = 2 * ntp + k
                                if dch == 0:
                                    nc.tensor.matmul(
                                        ps_sc,
                                        keep1c_t[:, nt : nt + 1],
                                        ft_s[:, k, :],
                                        start=(nt == 0),
                                        stop=(nt == NT - 1),
                                    )
                                for j in range(4):
                                    nc.tensor.matmul(
                                        psv[j],
                                        ft_s[:, k, j * P : (j + 1) * P],
                                        v_t[:, k, :],
                                        start=(nt == 0),
                                        stop=False,
                                    )
                        if dch == 0:
                            scol_row = scr.tile([1, 512], F32, tag="scrow")
                            nc.vector.tensor_copy(scol_row, ps_sc)
                            scraw = sml.tile([P, 4], F32, tag="scraw")
                            for j in range(4):
                                mt = q * 4 + j
                                ps_c = psum.tile([P, 1], F32, tag="ps")
                                nc.tensor.matmul(
                                    ps_c,
                                    scol_row[0:1, j * P : (j + 1) * P],
                                    one11_t,
                                    start=True,
                                    stop=False,
                                )
                                nc.tensor.matmul(
                                    ps_c,
                                    m2i_t[0:1, mt * P : (mt + 1) * P],
                                    c2048_t,
                                    start=False,
                                    stop=True,
                                    skip_group_check=True,
                                )
                                nc.vector.tensor_copy(scraw[:, j : j + 1], ps_c)
                            nc.vector.reciprocal(
                                scol_rec[:, q * 4 : (q + 1) * 4], scraw
                            )
                        for j in range(4):
                            mt = q * 4 + j
                            nc.tensor.matmul(
                                psv[j],
                                m2i_t[0:1, mt * P : (mt + 1) * P],
                                blr_t[0:1, dch * 512 : (dch + 1) * 512],
                                start=False,
                                stop=True,
                                skip_group_check=True,
                            )
                            st = stg.tile([P, 512], F32, tag="stage")
                            nc.vector.tensor_scalar(
                                out=st,
                                in0=psv[j],
                                scalar1=scol_rec[:, mt : mt + 1],
                                scalar2=None,
                                op0=Mult,
                            )
                            nc.sync.dma_start(
                                out=outb.ap()[
                                    b : b + 1,
                                    mt : mt + 1,
                                    :,
                                    dch * 512 : (dch + 1) * 512,
                                ].rearrange("o t p d -> p (o t d)"),
                                in_=st,
                            )


_NC_CACHE = None


def _get_nc():
    global _NC_CACHE
    if _NC_CACHE is None:
        nc = bacc.Bacc("TRN2", target_bir_lowering=False, debug=False)
        _emit(nc)
        nc.compile()
        _NC_CACHE = nc
    return _NC_CACHE


def _prep_in_maps(x1, x1_mask, x2, x2_mask, W1, b1, W2, b2):
    f32 = np.float32
    x1 = np.ascontiguousarray(x1, f32)
    x2 = np.ascontiguousarray(x2, f32)
    W1 = np.ascontiguousarray(W1, f32)
    W2 = np.ascontiguousarray(W2, f32)
    b1 = np.asarray(b1, f32)
    b2 = np.asarray(b2, f32)
    m1 = np.asarray(x1_mask, bool)
    m2 = np.asarray(x2_mask, bool)

    w1t = np.ascontiguousarray(W1.T).reshape(DT, P, D)
    w2t = np.ascontiguousarray(W2.T).reshape(DT, P, D)
    b1c = np.ascontiguousarray(b1.reshape(ET, P).T)
    b2c = np.ascontiguousarray(b2.reshape(ET, P).T)
    ident = np.eye(P, dtype=BF16_NP)
    onescol = np.ones((P, 1), BF16_NP)
    one11 = np.ones((1, 1), f32)
    c2048 = np.full((1, 1), 2048.0, BF16_NP)

    in_maps = []
    for c in range(NCORES):
        sl = slice(c * BPC, (c + 1) * BPC)
        x1c, x2c = x1[sl], x2[sl]
        m1c, m2c = m1[sl], m2[sl]
        x1tc = np.ascontiguousarray(x1c.transpose(0, 2, 1)).reshape(BPC, DT, P, Nn)
        x2tc = np.ascontiguousarray(x2c.transpose(0, 2, 1)).reshape(BPC, DT, P, Mm)
        x1z = np.where(m1c[:, :, None], 0.0, x1c).astype(BF16_NP)
        x1bc = np.ascontiguousarray(x1z).reshape(BPC, NT, P, D)
        x2bc = np.ascontiguousarray(x2c.astype(BF16_NP)).reshape(BPC, MT, P, D)
        x2mb = np.where(m2c, np.float64(NEG), 0.0) - C_SHIFT
        x2mbc = np.ascontiguousarray(
            x2mb.astype(f32).reshape(BPC, MT, P).transpose(0, 2, 1)
        )
        keep1 = (~m1c).astype(BF16_NP)
        keep1c = np.ascontiguousarray(keep1.reshape(BPC, NT, P).transpose(0, 2, 1))
        m2i = m2c.astype(BF16_NP).reshape(BPC, 1, Mm)
        blrow = x1c.sum(axis=1, dtype=np.float64).astype(BF16_NP).reshape(BPC, 1, D)
        in_maps.append(
            {
                "x1t": x1tc,
                "x2t": x2tc,
                "w1t": w1t,
                "w2t": w2t,
                "b1c": b1c,
                "b2c": b2c,
                "x1b": x1bc,
                "x2b": x2bc,
                "x2mbc": x2mbc,
                "keep1c": keep1c,
                "m2i": m2i,
                "blr": blrow,
                "ident": ident,
                "onescol": onescol,
                "one11": one11,
                "c2048": c2048,
            }
        )
    return in_maps


def kernel(x1, x1_mask, x2, x2_mask, W1, b1, W2, b2, _trace=False):
    nc = _get_nc()
    in_maps = _prep_in_maps(x1, x1_mask, x2, x2_mask, W1, b1, W2, b2)
    res = run_bass_kernel_spmd(nc, in_maps, core_ids=list(range(NCORES)), trace=_trace)
    attn_a = np.empty((B, Nn, D), np.float32)
    attn_b = np.empty((B, Mm, D), np.float32)
    for c in range(NCORES):
        sl = slice(c * BPC, (c + 1) * BPC)
        attn_a[sl] = res.results[c]["outa"].reshape(BPC, Nn, D)
        attn_b[sl] = res.results[c]["outb"].reshape(BPC, Mm, D)
    if _trace:
        kernel._last_exec_time_ns = res.exec_time_ns
        kernel._last_results = res
    return attn_a, attn_b
